# revision 14
# baseline (speedup 1.0000x reference)
"""Hyperbolic GNN classifier on 8 Trainium2 NeuronCores (Bass/Tile).

Only B=64 output rows are consumed (h2[to_fetch + 64*arange]), so the
kernel computes just the dependency cone of those rows: 8 outputs per
core -> 128 layer-1 aggregation instances -> 2048 feature rows. Each
core is fully independent (no collectives): it receives the full
feature/src_idx tables in DRAM and gathers what it needs.

Engine notes: the Scalar engine only ever evaluates {Square, Tanh,
Relu, Copy, Identity} so it stays on one activation table set (every
ACT_TABLE_LOAD switch costs ~2.7us). arctanh(n)/n is a Taylor series
in n^2 (norms < 0.45), sqrt/rsqrt is the bit-trick Newton rsqrt on the
Vector engine. Matmuls and the Mobius-fold accumulators run in fp16
(values ~1e-2, gate 2e-2; DVE accum_out taps the internal fp32
datapath so tiny fp16 products still accumulate exactly); per-row
norms stay fp32, packed as two fp16 columns via bitcast.
"""

import os

import numpy as np

import concourse.bass as bass
import concourse.bacc as bacc
import concourse.mybir as mybir
import concourse.tile as tile
from concourse.bass_utils import run_bass_kernel_spmd
from concourse.masks import make_identity

f32 = mybir.dt.float32
f16 = mybir.dt.float16
i32 = mybir.dt.int32
u32 = mybir.dt.uint32
OP = mybir.AluOpType
AF = mybir.ActivationFunctionType

NCORES = 8
N = 65536
KDEG = 16
DIN = 512
DH = 256
DOUT = 64
B = 64
P = 128
NPC = B // NCORES          # 8 outputs per core
EPS = 1e-7
MAXN = 1.0 - 1e-5
NORM = float(KDEG) ** -0.5  # 0.25
TG = 4                      # tiles per transform chain sub-batch
MAGIC = 0x5F3759DF
DW = DH + 2                 # fp16 row width incl. packed f32 norm

TRACE = False
LAST_RESULT = None
LAST_EXEC_NS = None
KDEBUG = bool(int(os.environ.get("KDEBUG", "0")))
_NC_CACHE = {}


def _rsqrt(nc, pool, cst, in_ap, p, tag, g=1, iters=2):
    """1/sqrt(x) on DVE: bit-trick seed + Newton (rel err ~6e-6 at 2
    iters; 1 iter's ~1.7e-3 systematic bias compounds across sites).
    Safe at x=0 (large finite result, downstream mults by 0)."""
    V = nc.vector
    ones, magic = cst
    h = pool.tile([p, g], u32, tag=f"rs_h{tag}")
    V.tensor_tensor(out=h[:], in0=in_ap.bitcast(u32), in1=ones[0:p, 0:g],
                    op=OP.logical_shift_right)
    y0u = pool.tile([p, g], u32, tag=f"rs_y0{tag}")
    V.tensor_tensor(out=y0u[:], in0=magic[0:p, 0:g], in1=h[:],
                    op=OP.subtract)
    y = y0u[:].bitcast(f32)
    for it in range(iters):
        t1 = pool.tile([p, g], f32, tag=f"rs_t1{tag}_{it}")
        V.tensor_tensor(out=t1[:], in0=y, in1=y, op=OP.mult)
        t2 = pool.tile([p, g], f32, tag=f"rs_t2{tag}_{it}")
        V.tensor_tensor(out=t2[:], in0=t1[:], in1=in_ap, op=OP.mult)
        t3 = pool.tile([p, g], f32, tag=f"rs_t3{tag}_{it}")
        V.tensor_scalar(out=t3[:], in0=t2[:], scalar1=-0.5, scalar2=1.5,
                        op0=OP.mult, op1=OP.add)
        yn = pool.tile([p, g], f32, tag=f"rs_y{tag}_{it}")
        V.tensor_tensor(out=yn[:], in0=y, in1=t3[:], op=OP.mult)
        y = yn[:]
    return y


# arctanh(sqrt(w))/sqrt(w) = 1 + w/3 + w^2/5 + ... ; Horner coeffs
_ATH_C = [1.0 / 11, 1.0 / 9, 1.0 / 7, 1.0 / 5, 1.0 / 3, 1.0]


def _atanh_series_col(nc, pool, w_ap, p, tag, nterms=6):
    """A(w) = arctanh(sqrt(w))/sqrt(w) for a [p, 1] column."""
    V = nc.vector
    cs = _ATH_C[-nterms:]
    h = pool.tile([p, 1], f32, tag=f"at_h0{tag}")
    V.tensor_scalar(out=h[:], in0=w_ap, scalar1=cs[0],
                    scalar2=cs[1], op0=OP.mult, op1=OP.add)
    for i, c in enumerate(cs[2:]):
        hn = pool.tile([p, 1], f32, tag=f"at_h{i + 1}{tag}")
        V.tensor_scalar(out=hn[:], in0=h[:], scalar1=w_ap, scalar2=c,
                        op0=OP.mult, op1=OP.add)
        h = hn
    return h


def _mstep(nc, pool, acc_ap, x2_ap, y_ap, y2_ap, py2_ap, cbn, p,
           width=DH, dt=f16):
    """acc <- mobius_add(acc, y) on [p, width] (dtype dt); x2_ap [p,1]
    f32 holds ||acc||^2 and is kept exact via ACT square-accum.
    py2_ap = 1+||y||^2; cbn = (1-||acc||^2) from the previous step.
    Returns next cbn. den's EPS clamp dropped (den > 0.3 always)."""
    V = nc.vector
    S = nc.scalar
    t0 = pool.tile([p, 1], f32, tag="ms_t0")
    V.tensor_tensor(out=t0[:], in0=x2_ap, in1=y2_ap, op=OP.mult)
    prod = pool.tile([p, width], dt, tag="ms_prod")
    xy = pool.tile([p, 1], f32, tag="ms_xy")
    V.scalar_tensor_tensor(out=prod[:], in0=acc_ap, scalar=1.0,
                           in1=y_ap, op0=OP.mult, op1=OP.mult,
                           accum_out=xy[:])
    # can = 1 + 2xy + y2 on ACT (off critical path)
    can = pool.tile([p, 1], f32, tag="ms_can")
    S.activation(can[:], xy[:], AF.Identity, bias=py2_ap, scale=2.0)
    u = pool.tile([p, 1], f32, tag="ms_u")
    V.tensor_scalar(out=u[:], in0=xy[:], scalar1=2.0, scalar2=1.0,
                    op0=OP.mult, op1=OP.add)
    den = pool.tile([p, 1], f32, tag="ms_den")
    V.tensor_tensor(out=den[:], in0=u[:], in1=t0[:], op=OP.add)
    r = pool.tile([p, 1], f32, tag="ms_r")
    V.reciprocal(r[:], den[:])
    cbr = pool.tile([p, 1], f32, tag="ms_cbr")
    V.tensor_tensor(out=cbr[:], in0=cbn[:], in1=r[:], op=OP.mult)
    t1_ = pool.tile([p, width], dt, tag="ms_t1")
    V.tensor_scalar(out=t1_[:], in0=acc_ap, scalar1=can[:, 0:1],
                    scalar2=r[:, 0:1], op0=OP.mult, op1=OP.mult)
    V.scalar_tensor_tensor(out=acc_ap, in0=y_ap,
                           scalar=cbr[:, 0:1], in1=t1_[:], op0=OP.mult,
                           op1=OP.add)
    sq = pool.tile([p, width], dt, tag="ms_sq")
    S.activation(sq[:], acc_ap, AF.Square, accum_out=x2_ap)
    cbn_n = pool.tile([p, 1], f32, tag="ms_cbn")
    S.activation(cbn_n[:], x2_ap, AF.Copy, bias=1.0, scale=-1.0)
    return cbn_n


def _cbn_of(nc, pool, x2_ap, p, tag="cb0"):
    cbn = pool.tile([p, 1], f32, tag=f"ms_{tag}")
    nc.scalar.activation(cbn[:], x2_ap, AF.Copy, bias=1.0, scale=-1.0)
    return cbn


def _act_store(nc, pool, cst, acc_ap, x2_ap, out_ap, outn2_ap, scale, p,
               width=DH):
    """out = scale * expmap0(relu(logmap0(acc))); outn2_ap (f32) gets
    ||out||^2."""
    V = nc.vector
    S = nc.scalar
    z = pool.tile([p, width], f32, tag="as_z")
    S.activation(z[:], acc_ap, AF.Relu)
    zsq = pool.tile([p, width], f32, tag="as_zsq")
    rn2 = pool.tile([p, 1], f32, tag="as_rn2")
    S.activation(zsq[:], z[:], AF.Square, accum_out=rn2[:])
    s = _atanh_series_col(nc, pool, x2_ap, p, "as", nterms=4)
    rr = _rsqrt(nc, pool, cst, rn2[:], p, "as")
    rnr = pool.tile([p, 1], f32, tag="as_rnr")
    V.tensor_tensor(out=rnr[:], in0=rn2[:], in1=rr, op=OP.mult)
    rarg = pool.tile([p, 1], f32, tag="as_rarg")
    V.tensor_tensor(out=rarg[:], in0=s[:], in1=rnr[:], op=OP.mult)
    gt = pool.tile([p, 1], f32, tag="as_gt")
    S.activation(gt[:], rarg[:], AF.Tanh)
    gam = pool.tile([p, 1], f32, tag="as_gam")
    V.tensor_tensor(out=gam[:], in0=gt[:], in1=rr, op=OP.mult)
    gams = pool.tile([p, 1], f32, tag="as_gams")
    V.tensor_scalar(out=gams[:], in0=gam[:], scalar1=scale, scalar2=None,
                    op0=OP.mult)
    V.tensor_scalar(out=out_ap, in0=z[:], scalar1=gams[:, 0:1],
                    scalar2=None, op0=OP.mult)
    gg = pool.tile([p, 1], f32, tag="as_gg")
    V.tensor_tensor(out=gg[:], in0=gams[:], in1=gams[:], op=OP.mult)
    V.tensor_tensor(out=outn2_ap, in0=gg[:], in1=rn2[:], op=OP.mult)


def _mobius_matvec(nc, pool, cst, ps, psmx, x_ap, xn2_ap, Wsb,
                   nchunk, dout, p, out, outn2, identh):
    """out = mobius_matvec(x, W) on [p, nchunk*128] (fp16) ->
    [p, dout]; outn2 (f32) gets tanh(r)^2. Wsb fp16."""
    V = nc.vector
    S = nc.scalar
    xT = pool.tile([P, nchunk, p], f16, tag="mv_xT")
    for c in range(nchunk):
        pt = ps.tile([P, P], f16, tag="pth")
        nc.tensor.transpose(out=pt[:, 0:p], in_=x_ap[:, c * P : (c + 1) * P],
                            identity=identh[0:p, 0:p])
        if c % 2 == 0:
            V.tensor_copy(out=xT[:, c, :], in_=pt[:, 0:p])
        else:
            S.activation(xT[:, c, :], pt[:, 0:p], AF.Copy)
    pm = psmx.tile([p, dout], f32, tag="pmx")
    for c in range(nchunk):
        nc.tensor.matmul(out=pm[:], lhsT=xT[:, c, :], rhs=Wsb[:, c, :],
                         start=(c == 0), stop=(c == nchunk - 1))
    msq = pool.tile([p, dout], f32, tag="mv_msq")
    mxn2 = pool.tile([p, 1], f32, tag="mv_mxn2")
    S.activation(msq[:], pm[:], AF.Square, accum_out=mxn2[:])
    at = _atanh_series_col(nc, pool, xn2_ap, p, "mv", nterms=6)
    rmx = _rsqrt(nc, pool, cst, mxn2[:], p, "mv")
    mxn = pool.tile([p, 1], f32, tag="mv_mxn")
    V.tensor_tensor(out=mxn[:], in0=mxn2[:], in1=rmx, op=OP.mult)
    rarg = pool.tile([p, 1], f32, tag="mv_rarg")
    V.tensor_tensor(out=rarg[:], in0=mxn[:], in1=at[:], op=OP.mult)
    th = pool.tile([p, 1], f32, tag="mv_th")
    S.activation(th[:], rarg[:], AF.Tanh)
    srow = pool.tile([p, 1], f32, tag="mv_srow")
    V.tensor_tensor(out=srow[:], in0=th[:], in1=rmx, op=OP.mult)
    V.tensor_scalar(out=out, in0=pm[:], scalar1=srow[:, 0:1],
                    scalar2=None, op0=OP.mult)
    S.activation(outn2, th[:], AF.Square)


def _build_nc():
    nc = bacc.Bacc("TRN2", target_bir_lowering=False, debug=False,
                   num_devices=NCORES)
    feat = nc.dram_tensor("feat", [N, DIN], f32, kind="ExternalInput")
    srcs = nc.dram_tensor("srcs", [N, KDEG], i32, kind="ExternalInput")
    sel = nc.dram_tensor("sel", [NPC, 1], i32, kind="ExternalInput")
    W1 = nc.dram_tensor("W1", [DIN, DH], f32, kind="ExternalInput")
    b1 = nc.dram_tensor("b1", [1, DH], f32, kind="ExternalInput")
    W2 = nc.dram_tensor("W2", [DH, DH], f32, kind="ExternalInput")
    b2 = nc.dram_tensor("b2", [1, DH], f32, kind="ExternalInput")
    WlT = nc.dram_tensor("WlT", [DH, DOUT], f32, kind="ExternalInput")
    bl = nc.dram_tensor("bl", [1, DOUT], f32, kind="ExternalInput")
    out = nc.dram_tensor("out", [NPC, DOUT], f32, kind="ExternalOutput")
    if KDEBUG:
        dbg_s2 = nc.dram_tensor("dbg_s2", [P, KDEG], i32,
                                kind="ExternalOutput")

    with tile.TileContext(nc) as tc:
        with (
            tc.tile_pool(name="sb", bufs=4) as sb,
            tc.tile_pool(name="ch", bufs=4) as ch,
            tc.tile_pool(name="wt", bufs=1) as wt,
            tc.tile_pool(name="yt", bufs=1) as yt,
            tc.tile_pool(name="ps", bufs=2, space="PSUM") as ps,
            tc.tile_pool(name="psmx", bufs=4, space="PSUM") as psmx,
            tc.tile_pool(name="dr", bufs=1, space="DRAM") as dr,
        ):
            # --- index chain first (scalar-engine HWDGE ring) ---
            selt = wt.tile([NPC, 1], i32, tag="selt")
            nc.scalar.dma_start(selt[:], sel[:])
            sidx1 = wt.tile([NPC, KDEG], i32, tag="sidx1")
            nc.gpsimd.indirect_dma_start(
                out=sidx1[:], out_offset=None, in_=srcs[:],
                in_offset=bass.IndirectOffsetOnAxis(ap=selt[:, 0:1], axis=0))
            s1d = dr.tile([P, 1], i32, tag="s1d")
            nc.scalar.dma_start(
                s1d[:].rearrange("(q k) one -> q (k one)", q=NPC), sidx1[:])
            s1sb = wt.tile([P, 1], i32, tag="s1sb")
            nc.scalar.dma_start(s1sb[:], s1d[:])
            sidx2 = wt.tile([P, KDEG], i32, tag="sidx2")
            nc.gpsimd.indirect_dma_start(
                out=sidx2[:], out_offset=None, in_=srcs[:],
                in_offset=bass.IndirectOffsetOnAxis(ap=s1sb[:, 0:1], axis=0))
            if KDEBUG:
                nc.sync.dma_start(dbg_s2[:], sidx2[:])

            ident = wt.tile([P, P], f32, tag="ident")
            make_identity(nc, ident[:])
            identh = wt.tile([P, P], f16, tag="identh")
            nc.vector.tensor_copy(out=identh[:], in_=ident[:])
            onesu = wt.tile([P, TG], u32, tag="onesu")
            nc.vector.memset(onesu[:], 1)
            magicu = wt.tile([P, TG], u32, tag="magicu")
            nc.vector.memset(magicu[:], MAGIC)
            cst = (onesu, magicu)

            # --- weights via HWDGE f32, DVE-cast to fp16 ---
            W1f = wt.tile([P, 4, DH], f32, tag="W1f")
            nc.sync.dma_start(
                W1f[:], W1[:].rearrange("(a p) d -> p a d", p=P))
            W1sb = wt.tile([P, 4, DH], f16, tag="W1sb")
            nc.vector.tensor_copy(out=W1sb[:], in_=W1f[:])
            W2f = wt.tile([P, 2, DH], f32, tag="W2f")
            nc.sync.dma_start(
                W2f[:], W2[:].rearrange("(a p) d -> p a d", p=P))
            W2sb = wt.tile([P, 2, DH], f16, tag="W2sb")
            nc.vector.tensor_copy(out=W2sb[:], in_=W2f[:])
            Wlf = wt.tile([P, 2, DOUT], f32, tag="Wlf")
            nc.sync.dma_start(
                Wlf[:], WlT[:].rearrange("(a p) d -> p a d", p=P))
            Wlsb = wt.tile([P, 2, DOUT], f16, tag="Wlsb")
            nc.vector.tensor_copy(out=Wlsb[:], in_=Wlf[:])
            b1row = wt.tile([1, DH], f32, tag="b1row")
            nc.sync.dma_start(b1row[:], b1[:])
            b1b = wt.tile([P, DH], f32, tag="b1b")
            nc.gpsimd.partition_broadcast(b1b[:], b1row[:])
            b1bh = wt.tile([P, DH], f16, tag="b1bh")
            nc.vector.tensor_copy(out=b1bh[:], in_=b1b[:])
            b2row = wt.tile([1, DH], f32, tag="b2row")
            nc.sync.dma_start(b2row[:], b2[:])
            b2b = wt.tile([NPC, DH], f32, tag="b2b")
            nc.gpsimd.partition_broadcast(b2b[:], b2row[:], channels=NPC)
            b2bh = wt.tile([NPC, DH], f16, tag="b2bh")
            nc.vector.tensor_copy(out=b2bh[:], in_=b2b[:])
            blrow = wt.tile([1, DOUT], f32, tag="blrow")
            nc.sync.dma_start(blrow[:], bl[:])
            blb = wt.tile([NPC, DOUT], f32, tag="blb")
            nc.gpsimd.partition_broadcast(blb[:], blrow[:], channels=NPC)
            bscr = wt.tile([P, DH], f32, tag="bscr")
            b1n2 = wt.tile([P, 1], f32, tag="b1n2")
            nc.scalar.activation(bscr[:], b1b[:], AF.Square,
                                 accum_out=b1n2[:])
            pb1n2 = wt.tile([P, 1], f32, tag="pb1n2")
            nc.vector.tensor_scalar(out=pb1n2[:], in0=b1n2[:], scalar1=1.0,
                                    scalar2=None, op0=OP.add)
            bscr2 = wt.tile([NPC, DH], f32, tag="bscr2")
            b2n2 = wt.tile([NPC, 1], f32, tag="b2n2")
            nc.scalar.activation(bscr2[:], b2b[:], AF.Square,
                                 accum_out=b2n2[:])
            pb2n2 = wt.tile([NPC, 1], f32, tag="pb2n2")
            nc.vector.tensor_scalar(out=pb2n2[:], in0=b2n2[:], scalar1=1.0,
                                    scalar2=None, op0=OP.add)
            bscr3 = wt.tile([NPC, DOUT], f32, tag="bscr3")
            bln2 = wt.tile([NPC, 1], f32, tag="bln2")
            nc.scalar.activation(bscr3[:], blb[:], AF.Square,
                                 accum_out=bln2[:])
            pbln2 = wt.tile([NPC, 1], f32, tag="pbln2")
            nc.vector.tensor_scalar(out=pbln2[:], in0=bln2[:], scalar1=1.0,
                                    scalar2=None, op0=OP.add)

            # --- feature gathers (fp16 cast) + layer-1 transform ---
            ytiles = [yt.tile([P, DH], f16, name=f"ytile{k}", tag=f"y{k}")
                      for k in range(KDEG)]
            y2g = [yt.tile([P, TG], f32, name=f"y2g{gi}", tag=f"y2g{gi}")
                   for gi in range(KDEG // TG)]
            py2g = [yt.tile([P, TG], f32, name=f"py2g{gi}", tag=f"pyg{gi}")
                    for gi in range(KDEG // TG)]
            for g0 in range(0, KDEG, TG):
                gi = g0 // TG
                xn2 = ch.tile([P, TG], f32, tag="tf_xn2")
                mxn2 = ch.tile([P, TG], f32, tag="tf_mxn2")
                pmx_list = []
                for j in range(TG):
                    k = g0 + j
                    ft = sb.tile([P, DIN], f16, tag="ft")
                    nc.gpsimd.indirect_dma_start(
                        out=ft[:], out_offset=None, in_=feat[:],
                        in_offset=bass.IndirectOffsetOnAxis(
                            ap=sidx2[:, k : k + 1], axis=0))
                    sq = sb.tile([P, DIN], f32, tag="sq")
                    nc.scalar.activation(sq[:], ft[:], AF.Square,
                                         accum_out=xn2[:, j : j + 1])
                    xT = sb.tile([P, 4, P], f16, tag="xT")
                    for c in range(4):
                        pt = ps.tile([P, P], f16, tag="pth")
                        nc.tensor.transpose(
                            out=pt[:], in_=ft[:, c * P : (c + 1) * P],
                            identity=identh[:])
                        if c % 2 == 0:
                            nc.vector.tensor_copy(out=xT[:, c, :],
                                                  in_=pt[:])
                        else:
                            nc.scalar.activation(xT[:, c, :], pt[:],
                                                 AF.Copy)
                    pmx = psmx.tile([P, DH], f32, tag="pmx")
                    for c in range(4):
                        nc.tensor.matmul(out=pmx[:], lhsT=xT[:, c, :],
                                         rhs=W1sb[:, c, :],
                                         start=(c == 0), stop=(c == 3))
                    pmx_list.append(pmx)
                # batched mobius_matvec chain over TG tiles
                for j in range(TG):
                    msq = sb.tile([P, DH], f32, tag="msq")
                    nc.scalar.activation(msq[:], pmx_list[j][:], AF.Square,
                                         accum_out=mxn2[:, j : j + 1])
                # at = A(NORM^2 * xn2); 2-term series (w ~ 5e-6 here)
                at = ch.tile([P, TG], f32, tag="tf_at")
                nc.vector.tensor_scalar(out=at[:], in0=xn2[:],
                                        scalar1=NORM * NORM / 3.0,
                                        scalar2=1.0, op0=OP.mult,
                                        op1=OP.add)
                rmx = _rsqrt(nc, ch, cst, mxn2[:], P, "tf", g=TG)
                mxn = ch.tile([P, TG], f32, tag="tf_mxn")
                nc.vector.tensor_tensor(out=mxn[:], in0=mxn2[:], in1=rmx,
                                        op=OP.mult)
                r2 = ch.tile([P, TG], f32, tag="tf_r2")
                nc.vector.scalar_tensor_tensor(out=r2[:], in0=mxn[:],
                                               scalar=NORM, in1=at[:],
                                               op0=OP.mult, op1=OP.mult)
                th = ch.tile([P, TG], f32, tag="tf_th")
                nc.scalar.activation(th[:], r2[:], AF.Tanh)
                srow = ch.tile([P, TG], f32, tag="tf_srow")
                nc.vector.tensor_tensor(out=srow[:], in0=th[:], in1=rmx,
                                        op=OP.mult)
                nc.scalar.activation(y2g[gi][:], th[:], AF.Square)
                nc.vector.tensor_scalar(out=py2g[gi][:], in0=y2g[gi][:],
                                        scalar1=1.0, scalar2=None,
                                        op0=OP.add)
                for j in range(TG):
                    k = g0 + j
                    nc.vector.tensor_scalar(
                        out=ytiles[k][:], in0=pmx_list[j][:],
                        scalar1=srow[:, j : j + 1], scalar2=None, op0=OP.mult)

            # --- layer-1 fold (15 steps on [128, 256] fp16) ---
            acc1 = yt.tile([P, DW], f16, tag="acc1")
            a1x2 = acc1[:, DH : DH + 2].bitcast(f32)
            nc.vector.tensor_copy(out=acc1[:, 0:DH], in_=ytiles[0][:])
            nc.vector.tensor_copy(out=a1x2, in_=y2g[0][:, 0:1])
            cbn = _cbn_of(nc, ch, a1x2, P)
            for k in range(1, KDEG):
                gi, j = k // TG, k % TG
                cbn = _mstep(nc, ch, acc1[:, 0:DH], a1x2, ytiles[k][:],
                             y2g[gi][:, j : j + 1],
                             py2g[gi][:, j : j + 1], cbn, P)
            # rst *= NORM (norm col by NORM^2)
            nc.vector.tensor_scalar(out=acc1[:, 0:DH], in0=acc1[:, 0:DH],
                                    scalar1=NORM, scalar2=None, op0=OP.mult)
            nc.vector.tensor_scalar(out=a1x2, in0=a1x2,
                                    scalar1=NORM * NORM, scalar2=None,
                                    op0=OP.mult)
            cbn = _cbn_of(nc, ch, a1x2, P, tag="cb1")
            _mstep(nc, ch, acc1[:, 0:DH], a1x2, b1bh[:], b1n2[:, 0:1],
                   pb1n2[:, 0:1], cbn, P)
            h1t = yt.tile([P, DW], f16, tag="h1t")
            _act_store(nc, ch, cst, acc1[:, 0:DH], a1x2, h1t[:, 0:DH],
                       h1t[:, DH : DH + 2].bitcast(f32), NORM, P)

            # --- regroup [128] -> [8, 16] via DRAM roundtrip ---
            h1d = dr.tile([P, DW], f16, tag="h1d")
            nc.scalar.dma_start(h1d[:], h1t[:])
            h1r = yt.tile([NPC, KDEG, DW], f16, tag="h1r")
            nc.scalar.dma_start(
                h1r[:], h1d[:].rearrange("(q k) d -> q k d", q=NPC))

            # --- layer-2 fold on [8, 256] fp16 ---
            py2b = wt.tile([NPC, KDEG], f32, tag="py2b")
            nc.vector.tensor_scalar(
                out=py2b[:],
                in0=h1r[:, :, DH : DH + 2].bitcast(f32).rearrange(
                    "q k one -> q (k one)"),
                scalar1=1.0, scalar2=None, op0=OP.add)
            acc2 = yt.tile([NPC, DW], f16, tag="acc2")
            a2x2 = acc2[:, DH : DH + 2].bitcast(f32)
            nc.vector.tensor_copy(out=acc2[:, 0:DH], in_=h1r[:, 0, 0:DH])
            nc.vector.tensor_copy(out=a2x2,
                                  in_=h1r[:, 0, DH : DH + 2].bitcast(f32))
            cbn = _cbn_of(nc, ch, a2x2, NPC, tag="cb2")
            for k in range(1, KDEG):
                cbn = _mstep(nc, ch, acc2[:, 0:DH], a2x2,
                             h1r[:, k, 0:DH],
                             h1r[:, k, DH : DH + 2].bitcast(f32),
                             py2b[:, k : k + 1], cbn, NPC)
            # mobius_matvec W2
            v2 = yt.tile([NPC, DW], f16, tag="v2")
            v2x2 = v2[:, DH : DH + 2].bitcast(f32)
            _mobius_matvec(nc, ch, cst, ps, psmx, acc2[:, 0:DH],
                           a2x2, W2sb, 2, DH, NPC, v2[:, 0:DH], v2x2,
                           identh)
            nc.vector.tensor_scalar(out=v2[:, 0:DH], in0=v2[:, 0:DH],
                                    scalar1=NORM, scalar2=None, op0=OP.mult)
            nc.vector.tensor_scalar(out=v2x2, in0=v2x2,
                                    scalar1=NORM * NORM, scalar2=None,
                                    op0=OP.mult)
            cbn = _cbn_of(nc, ch, v2x2, NPC, tag="cb3")
            _mstep(nc, ch, v2[:, 0:DH], v2x2, b2bh[:], b2n2[:, 0:1],
                   pb2n2[:, 0:1], cbn, NPC)
            h2t = yt.tile([NPC, DW], f16, tag="h2t")
            h2x2 = h2t[:, DH : DH + 2].bitcast(f32)
            _act_store(nc, ch, cst, v2[:, 0:DH], v2x2, h2t[:, 0:DH],
                       h2x2, 1.0, NPC)

            # --- final mobius Linear 256 -> 64 + mobius_add(bl), f32 ---
            vf = yt.tile([NPC, DOUT + 1], f32, tag="vf")
            _mobius_matvec(nc, ch, cst, ps, psmx, h2t[:, 0:DH],
                           h2x2, Wlsb, 2, DOUT, NPC, vf[:, 0:DOUT],
                           vf[:, DOUT : DOUT + 1], identh)
            cbn = _cbn_of(nc, ch, vf[:, DOUT : DOUT + 1], NPC, tag="cb4")
            _mstep(nc, ch, vf[:, 0:DOUT], vf[:, DOUT : DOUT + 1], blb[:],
                   bln2[:, 0:1], pbln2[:, 0:1], cbn, NPC, width=DOUT,
                   dt=f32)
            outt = wt.tile([NPC, DOUT], f32, tag="outt")
            nc.vector.tensor_copy(out=outt[:], in_=vf[:, 0:DOUT])
            nc.sync.dma_start(out[:], outt[:])

    nc.compile()
    return nc


def _get_nc():
    if "nc" not in _NC_CACHE:
        _NC_CACHE["nc"] = _build_nc()
    return _NC_CACHE["nc"]


def kernel(features, W1, b1, W2, b2, Wl, bl, src_idx, to_fetch):
    global LAST_EXEC_NS, LAST_RESULT
    nc = _get_nc()
    features = np.ascontiguousarray(np.asarray(features, dtype=np.float32))
    src_idx = np.ascontiguousarray(np.asarray(src_idx, dtype=np.int32))
    to_fetch = np.asarray(to_fetch, dtype=np.int32)
    W1 = np.ascontiguousarray(np.asarray(W1, np.float32))
    b1 = np.asarray(b1, np.float32).reshape(1, DH)
    W2 = np.ascontiguousarray(np.asarray(W2, np.float32))
    b2 = np.asarray(b2, np.float32).reshape(1, DH)
    WlT = np.ascontiguousarray(np.asarray(Wl, np.float32).T)
    bl = np.asarray(bl, np.float32).reshape(1, DOUT)

    n_per = N // B
    in_maps = []
    for c in range(NCORES):
        bidx = np.arange(c * NPC, (c + 1) * NPC, dtype=np.int32)
        selv = (to_fetch[bidx] + bidx * n_per).astype(np.int32).reshape(
            NPC, 1)
        in_maps.append({
            "feat": features, "srcs": src_idx, "sel": selv,
            "W1": W1, "b1": b1, "W2": W2, "b2": b2, "WlT": WlT, "bl": bl,
        })
    res = run_bass_kernel_spmd(nc, in_maps, core_ids=list(range(NCORES)),
                               trace=TRACE)
    LAST_RESULT = res
    LAST_EXEC_NS = res.exec_time_ns
    return np.concatenate([res.results[c]["out"] for c in range(NCORES)],
                          axis=0)
